# revision 23
# baseline (speedup 1.0000x reference)
"""Multi-head attention (B=1, T=1500, N=1280, H=20, D=64) on 8 NeuronCores.

Tensor-parallel by head groups, 2.5 heads/core: core c owns full heads
F0=2c, F1=2c+1 plus half of shared head S=16+c//2 (q rows 750*(c%2)..+750).

Baseline bf16 dataflow (windows S, F0, F1 + drain; ACT-paced exp stream)
with the q/k/v PROJECTIONS done as fp8(e4m3) residual-3 DoubleRow
matmuls: a ~= a_h + a_l split on the host, products XhWh + XhWl + XlWh
in DoubleRow K=256 chunks at 0.5 cyc/col -> 0.75x the bf16 PE cost at
better-than-bf16 accuracy.  The F0/F1 windows are exp(ACT)-paced, so
only PE-paced phases profit from fp8: the projection front (S window)
and the drain.  x ships as xh+xl (same bytes as bf16); all input DMAs
ride one FIFO queue ordered [wqk pair, xh chunks, xl chunks, rest] so
the readiness-based scheduler cannot dilute the projection stream;
qs/k3 head the two other queues to start the S exp trickle at ~4us.

fp8 details that matter: dual-fp8 ldweights needs 64B-aligned slab
strides (x pair-chunks padded to stride 1536), and e4m3 goes subnormal
below 2^-6 (the QK_SCALE-folded k weights sit at sigma~0.004), so the
host pre-scales w by 128 and x by 8 and the psum->SBUF stage ops fold
1/1024 back in (activation Identity scale= / scalar_tensor_tensor).
m1's first chunk rides inside m0's x-chasing passes so K12 chunk 0 is
ready when the last xl chunk lands; bqk/bv head the short gpsimd DMA
queue (a late bias DMA once gated the whole F0 window).

Other structure is inherited from the bf16 predecessor:
  - software-pipelined by head in window order S, F0, F1,
  - shared head's small projections computed on the host and DMA'd in,
  - F0 hides the v-projection (2 slots x 66 cols incl. a ones column
    that makes wv emit the softmax denominator Z),
  - F1 hides wv of S and F0 (flipped [q, d] layout, deferred PE
    transposes), exp without max subtraction,
  - drain: F1's wv chain + all out-projections; bf16 output DMAs issue
    from SP; the 8 cores' partials are summed in jax.
"""

import os

import numpy as np

T, F, D = 1500, 1280, 64
NH = 20
QK_SCALE = D ** (-0.5)
NCORES = 8

PT = [128] * 11 + [92]          # partition tiles along t (sum = 1500)
PT_OFF = [128 * i for i in range(12)]
FULLSUB = [(0, 512), (512, 512), (1024, 476)]
SSUB = [(0, 512), (512, 238)]   # shared head q window = 750
SQT = 125                       # shared head q-tile width (6 x 125 = 750)
OCH = [(0, 512), (512, 512), (1024, 256)]
VSLOT = 66                      # 64 v cols + ones + pad
VW = 2 * VSLOT                  # 132 (two full-head slots)

_CACHE = {}
LAST_RESULTS = None


def _build():
    import concourse.bacc as bacc
    import concourse.mybir as mybir
    import concourse.tile as tile

    from concourse.alu_op_type import AluOpType

    f32 = mybir.dt.float32
    bf16 = mybir.dt.bfloat16
    e4 = mybir.dt.float8e4
    DR = mybir.MatmulPerfMode.DoubleRow
    IDF = mybir.ActivationFunctionType.Identity
    # fp8 pre-scales: w*128 / x*8 keep e4m3 operands out of the subnormal
    # range (QK_SCALE-folded k weights sit at sigma~0.004 unscaled); the
    # psum->SBUF stage ops fold the inverse back in.
    UNSCALE = 1.0 / 1024.0

    nc = bacc.Bacc("TRN2", target_bir_lowering=False, debug=False,
                   num_devices=NCORES)

    xh_d = nc.dram_tensor("xh", [F, T], e4, kind="ExternalInput").ap()
    xl_d = nc.dram_tensor("xl", [F, T], e4, kind="ExternalInput").ap()
    wqkh_d = nc.dram_tensor("wqkh", [F, 256], e4, kind="ExternalInput").ap()
    wqkl_d = nc.dram_tensor("wqkl", [F, 256], e4, kind="ExternalInput").ap()
    bqk_d = nc.dram_tensor("bqk", [128, 1], f32, kind="ExternalInput").ap()
    qs_d = nc.dram_tensor("qs", [64, 750], bf16, kind="ExternalInput").ap()
    k3_d = nc.dram_tensor("k3", [64, T], bf16, kind="ExternalInput").ap()
    v3_d = nc.dram_tensor("v3", [1536, VSLOT], bf16,
                          kind="ExternalInput").ap()
    wvwh_d = nc.dram_tensor("wvwh", [F, VW], e4, kind="ExternalInput").ap()
    wvwl_d = nc.dram_tensor("wvwl", [F, VW], e4, kind="ExternalInput").ap()
    bv_d = nc.dram_tensor("bv", [1, VW], f32, kind="ExternalInput").ap()
    wo_d = nc.dram_tensor("wo", [192, F], bf16, kind="ExternalInput").ap()
    idn_d = nc.dram_tensor("idn", [128, 128], bf16,
                           kind="ExternalInput").ap()
    out_d = nc.dram_tensor("out", [T, F], bf16, kind="ExternalOutput").ap()
    out2_d = nc.dram_tensor("out2", [750, F], bf16,
                            kind="ExternalOutput").ap()

    EXP = mybir.ActivationFunctionType.Exp

    import concourse.bass as bass

    with tile.TileContext(nc) as tc:
        from contextlib import ExitStack
        with ExitStack() as ctx:
            persist = ctx.enter_context(tc.tile_pool(name="persist", bufs=1))

            # ---------------- persistent SBUF ----------------
            # x pair-chunks: XH[j] [128, 3072]: cols 0:1500 = x rows
            # 256j..256j+128, cols 1536:3036 = rows 256j+128..256j+256
            # (slab stride 1536: dual-fp8 ldweights requires 64B-aligned
            # slab strides -- s3_lw_dual_fp8_restrictions)
            XSTR = 1536
            XH = [persist.tile([128, 2 * XSTR], e4, tag=f"xh{j}",
                               name=f"xh{j}") for j in range(5)]
            XL = [persist.tile([128, 2 * XSTR], e4, tag=f"xl{j}",
                               name=f"xl{j}") for j in range(5)]
            WQKH = persist.tile([128, 2560], e4, tag="wqkh", name="wqkh")
            WQKL = persist.tile([128, 2560], e4, tag="wqkl", name="wqkl")
            WVWH = persist.tile([128, 10 * VW], e4, tag="wvwh", name="wvwh")
            WVWL = persist.tile([128, 10 * VW], e4, tag="wvwl", name="wvwl")
            Q12 = persist.tile([128, T], bf16, tag="q12", name="q12")
            K12 = persist.tile([128, T], bf16, tag="k12", name="k12")
            K3 = persist.tile([64, T], bf16, tag="k3", name="k3")
            QS = persist.tile([64, 750], bf16, tag="qs", name="qs")
            V = persist.tile([128, 12 * VW], bf16, tag="vall", name="vall")
            V3 = persist.tile([128, 12 * VSLOT], bf16, tag="v3", name="v3")
            WVT12 = persist.tile([128, T], bf16, tag="wvt12", name="wvt12")
            WVT3 = persist.tile([64, 750], bf16, tag="wvt3", name="wvt3")
            WO12 = persist.tile([128, F], bf16, tag="wo12", name="wo12")
            WO3 = persist.tile([64, F], bf16, tag="wo3", name="wo3")
            BQK = persist.tile([128, 1], f32, tag="bqk", name="bqk")
            BV = persist.tile([128, VW], f32, tag="bv", name="bv")
            IDN = persist.tile([128, 128], bf16, tag="idn", name="idn")

            epool = ctx.enter_context(tc.tile_pool(name="epool", bufs=24))
            fin = ctx.enter_context(tc.tile_pool(name="fin", bufs=12))
            ost = ctx.enter_context(tc.tile_pool(name="ost", bufs=16))

            # ---------------- input DMAs ----------------
            # Transfers round-robin across the HWDGE queues by READINESS
            # (not emission), so qs/k3 head the two short queues (the S
            # trickle starts ~4us) and every other input rides the sync
            # FIFO in priority order: the projection stream (wqk pair,
            # xh chunks, xl chunks) cannot be diluted by later tensors.
            def batched_chunks(dst_tile, src, width, f0=0, nf=10,
                               eng=None):
                d0 = dst_tile[:, :]
                dst = bass.AP(tensor=d0.tensor,
                              offset=d0.offset + f0 * width,
                              ap=[list(d0.ap)[0], [width, nf], [1, width]])
                s0 = src[0:128, :]
                dsrc = bass.AP(tensor=s0.tensor,
                               offset=s0.offset + f0 * 128 * width,
                               ap=[list(s0.ap)[0], [128 * width, nf],
                                   [1, width]])
                (eng or nc.sync).dma_start(dst, dsrc)

            nc.gpsimd.dma_start(QS[:], qs_d[:, :])
            nc.gpsimd.dma_start(BQK[:], bqk_d[:, :])
            bvs = bv_d[0:1, :]
            bv_bc = bass.AP(tensor=bvs.tensor, offset=bvs.offset,
                            ap=[[0, 128]] + list(bvs.ap)[1:])
            nc.gpsimd.dma_start(BV[:], bv_bc)
            # k3 split: the first S score tile needs only cols 0:128, so a
            # small head transfer starts the exp trickle earlier
            nc.scalar.dma_start(K3[0:64, 0:256], k3_d[:, 0:256])
            nc.scalar.dma_start(K3[0:64, 256:T], k3_d[:, 256:T])
            batched_chunks(WQKH, wqkh_d, 256)
            batched_chunks(WQKL, wqkl_d, 256)

            def x_pair_dma(dst_tile, src, j):
                d0 = dst_tile[:, :]
                dst = bass.AP(tensor=d0.tensor, offset=d0.offset,
                              ap=[list(d0.ap)[0], [XSTR, 2], [1, T]])
                s0 = src[0:128, :]
                dsrc = bass.AP(tensor=s0.tensor,
                               offset=s0.offset + 256 * j * T,
                               ap=[list(s0.ap)[0], [128 * T, 2], [1, T]])
                nc.sync.dma_start(dst, dsrc)

            for j in range(5):
                x_pair_dma(XH[j], xh_d, j)
            for j in range(5):
                x_pair_dma(XL[j], xl_d, j)
            batched_chunks(WVWH, wvwh_d, VW)
            batched_chunks(WVWL, wvwl_d, VW)
            v3dst = V3[:, :]
            v3src = bass.AP(tensor=v3_d.tensor, offset=0,
                            ap=[[VSLOT, 128], [128 * VSLOT, 12], [1, VSLOT]])
            v3dst3 = bass.AP(tensor=v3dst.tensor, offset=v3dst.offset,
                             ap=[list(v3dst.ap)[0], [VSLOT, 12], [1, VSLOT]])
            nc.sync.dma_start(v3dst3, v3src)
            nc.sync.dma_start(IDN[:], idn_d[:, :])
            nc.sync.dma_start(WO12[:], wo_d[0:128, :])
            nc.sync.dma_start(WO3[:], wo_d[128:192, :])

            def ap3(tile_ap, counts_strides, off=0):
                return bass.AP(tensor=tile_ap.tensor,
                               offset=tile_ap.offset + off,
                               ap=[list(tile_ap.ap)[0]] +
                                  [[s, c] for (s, c) in counts_strides])

            # ---------------- pipelined attention ----------------
            att_ctx = ExitStack()
            sc_ctx = ExitStack()
            sc_pool = [sc_ctx.enter_context(
                tc.tile_pool(name="pp_scs", bufs=2, space="PSUM"))]
            qk_ctx = ExitStack()
            pp_qk = qk_ctx.enter_context(
                tc.tile_pool(name="pp_qk", bufs=4, space="PSUM",
                             side="right"))

            E12 = [[None] * 12 for _ in range(3)]
            pend = {}   # wv tiles awaiting their deferred transpose
            cp_rr = [0]
            att_pool = [None]
            wv_bufs = [1]
            pp_o = None

            def emit_scores(h, kt):
                pk = PT[kt]
                wq = T if h < 2 else 750
                ps = sc_pool[0].tile([128, wq], f32, tag="psc", name="psc")
                sub = FULLSUB if h < 2 else SSUB
                if h < 2:
                    lh = K12[64 * h:64 * h + 64, PT_OFF[kt]:PT_OFF[kt] + pk]
                    qsrc = Q12[64 * h:64 * h + 64, :]
                else:
                    lh = K3[0:64, PT_OFF[kt]:PT_OFF[kt] + pk]
                    qsrc = QS[0:64, :]
                for (o, w) in sub:
                    nc.tensor.matmul(
                        ps[0:pk, o:o + w], lh, qsrc[:, o:o + w],
                        start=True, stop=True,
                    )
                if h < 2:
                    E = epool.tile([128, T], bf16, tag="E", name="E")
                else:
                    E = epool.tile([128, 750], bf16, tag="ES", bufs=12,
                                   name="ES")
                nc.scalar.activation(E[0:pk, 0:wq], ps[0:pk, 0:wq], EXP)
                E12[h][kt] = E

            PRODS = ((XH, 0), (XH, 1), (XL, 0))  # (x level, w level)

            def emit_vproj(tt):
                pk = PT[tt]
                ps = pp_v.tile([128, VW], f32, tag="pv", name="pv")
                n = 0
                for (XS, wl) in PRODS:
                    WT = WVWH if wl == 0 else WVWL
                    for j in range(5):
                        lhsT = ap3(XS[j][:, :], [(XSTR, 2), (1, pk)],
                                   off=PT_OFF[tt])
                        rhs = ap3(WT[:, :], [(VW, 2), (1, VW)],
                                  off=2 * VW * j)
                        nc.tensor.matmul(
                            ps[0:pk, :], lhsT, rhs,
                            start=(n == 0), stop=(n == 14), perf_mode=DR,
                        )
                        n += 1
                nc.vector.scalar_tensor_tensor(
                    V[0:pk, VW * tt:VW * (tt + 1)], ps[0:pk, :], UNSCALE,
                    BV[0:pk, :], AluOpType.mult, AluOpType.add)

            def emit_wv(h, qt):
                """wv for head h, q-tile qt (kt-inner accumulation) plus the
                DVE normalize; the PE transpose is deferred to flush_norm."""
                if h < 2:
                    pkq, qo = PT[qt], PT_OFF[qt]
                else:
                    pkq, qo = SQT, SQT * qt
                ps = att_pool[0].tile([128, VSLOT], f32, tag="wv",
                                      bufs=wv_bufs[0], name="wv")
                for kt in range(12):
                    pkk = PT[kt]
                    if h < 2:
                        vsrc = V[0:pkk, VW * kt + VSLOT * h:
                                 VW * kt + VSLOT * h + VSLOT]
                    else:
                        vsrc = V3[0:pkk, VSLOT * kt:VSLOT * (kt + 1)]
                    nc.tensor.matmul(
                        ps[0:pkq, 0:VSLOT],
                        E12[h][kt][0:pkk, qo:qo + pkq],
                        vsrc,
                        start=(kt == 0), stop=(kt == 11),
                    )
                rz = fin.tile([128, 1], f32, tag="rz", name="rz")
                nc.vector.reciprocal(rz[0:pkq, 0:1], ps[0:pkq, 64:65])
                wsb = fin.tile([128, 64], bf16, tag="wsb", name="wsb")
                nc.vector.tensor_scalar_mul(wsb[0:pkq, 0:64],
                                            ps[0:pkq, 0:64], rz[0:pkq, 0:1])
                pend[(h, qt)] = (wsb, pkq, qo)

            def flush_norm(h, qt):
                """PE transpose + WVT copy for a pending wv tile."""
                wsb, pkq, qo = pend.pop((h, qt))
                pst = att_pool[0].tile([64, 128], bf16, tag="pt", name="pt")
                nc.tensor.transpose(pst[0:64, 0:pkq], wsb[0:pkq, 0:64],
                                    IDN[0:pkq, 0:pkq])
                # h=0/2 run inside exp windows: keep ACT free for exp.
                # h=1 runs in the drain where ACT has slack.
                cp = nc.scalar.copy if h == 1 else nc.vector.tensor_copy
                if h < 2:
                    cp(WVT12[64 * h:64 * h + 64, qo:qo + pkq],
                       pst[0:64, 0:pkq])
                else:
                    cp(WVT3[0:64, qo:qo + pkq], pst[0:64, 0:pkq])

            def emit_outproj(lhs, wo, dst, pk, dsl):
                stage = ost.tile([128, F], bf16, tag="stage", name="stage")
                ps = pp_o.tile([128, 1024], f32, tag="po", name="po")
                pst2 = att_pool[0].tile([128, 256], f32, tag="po256",
                                        name="po256")
                for (o, w) in ((0, 512), (512, 512)):
                    nc.tensor.matmul(
                        ps[0:pk, o:o + w], lhs, wo[:, o:o + w],
                        start=True, stop=True,
                    )
                nc.tensor.matmul(pst2[0:pk, 0:256], lhs, wo[:, 1024:1280],
                                 start=True, stop=True)
                cp_rr[0] += 1
                a, b = ((nc.scalar.copy, nc.vector.tensor_copy)
                        if cp_rr[0] % 2 else
                        (nc.vector.tensor_copy, nc.scalar.copy))
                a(stage[0:pk, 0:1024], ps[0:pk, 0:1024])
                b(stage[0:pk, 1024:1280], pst2[0:pk, 0:256])
                nc.sync.dma_start(dst[dsl, :], stage[0:pk, :])

            def emit_outproj12(tt):
                pk = PT[tt]
                tsl = slice(PT_OFF[tt], PT_OFF[tt] + pk)
                emit_outproj(WVT12[:, tsl], WO12, out_d, pk, tsl)

            def emit_outproj3(qt):
                qsl = slice(SQT * qt, SQT * (qt + 1))
                emit_outproj(WVT3[:, qsl], WO3, out2_d, SQT, qsl)

            # ---- window S: shared-head scores/exp, projection fillers ----
            for kt in range(12):
                emit_scores(2, kt)

            # m0 (q) + m1's FIRST chunk: res-3 DoubleRow in two passes
            # matching the x DMA order (Xh products chase the xh chunks,
            # then the Xl product chases xl).  Riding m1-c0 inside the
            # x-chase means k chunk 0 (and F0's first score tiles) is
            # ready right when the last xl chunk lands instead of two
            # chunk-times later.
            ps0 = [pp_qk.tile([128, 512], f32, tag="pqk", name="pqk",
                              bufs=4) for _ in range(3)]
            ps1c0 = pp_qk.tile([128, 512], f32, tag="pqk", name="pqk",
                               bufs=4)
            n = 0
            for pr in (((XH, 0), (XH, 1)), ((XL, 0),)):
                for j in range(5):
                    for (XS, wl) in pr:
                        WT = WQKH if wl == 0 else WQKL
                        lhsT = ap3(WT[:, :], [(256, 2), (1, 128)],
                                   off=512 * j)
                        for ci, (o, w) in enumerate(FULLSUB):
                            rhs = ap3(XS[j][:, :], [(XSTR, 2), (1, w)],
                                      off=o)
                            nc.tensor.matmul(ps0[ci][0:128, 0:w], lhsT, rhs,
                                             start=(n == 0), stop=(n == 14),
                                             perf_mode=DR)
                        lhsTk = ap3(WT[:, :], [(256, 2), (1, 128)],
                                    off=512 * j + 128)
                        rhsk = ap3(XS[j][:, :], [(XSTR, 2), (1, 512)],
                                   off=0)
                        nc.tensor.matmul(ps1c0[0:128, 0:512], lhsTk, rhsk,
                                         start=(n == 0), stop=(n == 14),
                                         perf_mode=DR)
                        n += 1
            nc.scalar.activation(K12[:, 0:512], ps1c0[0:128, 0:512],
                                 IDF, scale=UNSCALE)
            for ci, (o, w) in enumerate(FULLSUB):
                nc.scalar.activation(Q12[:, o:o + w], ps0[ci][0:128, 0:w],
                                     IDF, bias=BQK[:, 0:1], scale=UNSCALE)

            # m1 (k) remaining chunks: C-MAJOR res-3 with an inline copy
            # per chunk
            for ci, (o, w) in list(enumerate(FULLSUB))[1:]:
                ps1 = pp_qk.tile([128, 512], f32, tag="pqk", name="pqk",
                                 bufs=4)
                n = 0
                for (XS, wl) in PRODS:
                    WT = WQKH if wl == 0 else WQKL
                    for j in range(5):
                        lhsT = ap3(WT[:, :], [(256, 2), (1, 128)],
                                   off=512 * j + 128)
                        rhs = ap3(XS[j][:, :], [(XSTR, 2), (1, w)],
                                  off=o)
                        nc.tensor.matmul(ps1[0:128, 0:w], lhsT, rhs,
                                         start=(n == 0), stop=(n == 14),
                                         perf_mode=DR)
                        n += 1
                # k has no bias; the copy folds the fp8 pre-scale away
                nc.scalar.activation(K12[:, o:o + w], ps1[0:128, 0:w],
                                     IDF, scale=UNSCALE)
            qk_ctx.close()

            # ---- window F0: scores/exp + v-proj fillers ----
            sc_ctx.close()
            sc_ctx = ExitStack()
            sc_pool[0] = sc_ctx.enter_context(
                tc.tile_pool(name="pp_sc", bufs=2, space="PSUM"))
            vp_ctx = ExitStack()
            pp_v = vp_ctx.enter_context(
                tc.tile_pool(name="pp_v", bufs=2, space="PSUM",
                             side="right"))
            for kt in range(12):
                emit_scores(0, kt)
                emit_vproj(kt)
            vp_ctx.close()

            # ---- window F1: scores/exp + F0 wv + shared-head wv ----
            att_pool[0] = att_ctx.enter_context(
                tc.tile_pool(name="pp_att", bufs=1, space="PSUM",
                             side="right"))
            for kt in range(12):
                emit_scores(1, kt)
                emit_wv(0, kt)
                if kt % 2 == 1:
                    emit_wv(2, kt // 2)
                if kt >= 1:
                    flush_norm(0, kt - 1)
                if kt % 2 == 0 and kt >= 2:
                    flush_norm(2, kt // 2 - 1)
            flush_norm(0, 11)
            flush_norm(2, 5)

            # ---- drain: F1 wv + all out-projections ----
            sc_ctx.close()
            att_ctx.close()
            att_ctx = ExitStack()
            att_pool[0] = att_ctx.enter_context(
                tc.tile_pool(name="pp_att2", bufs=1, space="PSUM",
                             side="right"))
            wv_bufs[0] = 2
            o_ctx = ExitStack()
            pp_o = o_ctx.enter_context(
                tc.tile_pool(name="pp_o", bufs=2, space="PSUM"))
            for qt in range(12):
                emit_wv(1, qt)
                if qt >= 1:
                    flush_norm(1, qt - 1)
                if qt < 6:
                    emit_outproj3(qt)
            flush_norm(1, 11)
            for tt in range(12):
                emit_outproj12(tt)
            o_ctx.close()
            att_ctx.close()

    nc.compile()
    return nc


def _get_nc(_unused=None):
    if "nc" not in _CACHE:
        _CACHE["nc"] = _build()
    return _CACHE["nc"]


def _split8(a):
    import ml_dtypes
    a = np.asarray(a, dtype=np.float32)
    hi = a.astype(ml_dtypes.float8_e4m3)
    lo = (a - hi.astype(np.float32)).astype(ml_dtypes.float8_e4m3)
    return hi, lo


def _to_bf(a):
    import ml_dtypes
    return np.ascontiguousarray(np.asarray(a, dtype=np.float32)).astype(
        ml_dtypes.bfloat16)


def _prep_all(x, Wq, bq, Wk, Wv, bv, Wo):
    x = np.asarray(x, dtype=np.float32).reshape(T, F)
    xT = np.ascontiguousarray(x.T)
    xh, xl = _split8(xT * np.float32(8.0))
    WqT = np.ascontiguousarray(np.asarray(Wq, dtype=np.float32).T)
    WkTs = (np.ascontiguousarray(np.asarray(Wk, dtype=np.float32).T)
            * np.float32(QK_SCALE))
    WvT = np.ascontiguousarray(np.asarray(Wv, dtype=np.float32).T)
    WoT = np.ascontiguousarray(np.asarray(Wo, dtype=np.float32).T)
    bq = np.asarray(bq, dtype=np.float32)
    bvv = np.asarray(bv, dtype=np.float32)

    def hsl(h):
        return slice(D * h, D * (h + 1))

    in_maps = []
    for c in range(NCORES):
        F0, F1 = 2 * c, 2 * c + 1
        S = 16 + c // 2

        wqk = np.zeros((F, 256), dtype=np.float32)
        wqk[:, 0:64] = WqT[:, hsl(F0)]
        wqk[:, 64:128] = WqT[:, hsl(F1)]
        wqk[:, 128:192] = WkTs[:, hsl(F0)]
        wqk[:, 192:256] = WkTs[:, hsl(F1)]
        wqkh, wqkl = _split8(wqk * np.float32(128.0))

        bqk = np.zeros((128, 1), dtype=np.float32)
        bqk[0:64, 0] = bq[hsl(F0)]
        bqk[64:128, 0] = bq[hsl(F1)]

        roff = 750 * (c % 2)
        qs = (x[roff:roff + 750] @ WqT[:, hsl(S)] + bq[hsl(S)]).T
        k3 = (x @ WkTs[:, hsl(S)]).T
        v3 = np.zeros((1536, VSLOT), dtype=np.float32)
        v3[0:T, 0:64] = x @ WvT[:, hsl(S)] + bvv[hsl(S)]
        v3[0:T, 64] = 1.0

        wvw = np.zeros((F, VW), dtype=np.float32)
        bvr = np.zeros((1, VW), dtype=np.float32)
        for s, h in enumerate((F0, F1)):
            wvw[:, VSLOT * s:VSLOT * s + 64] = WvT[:, hsl(h)]
            bvr[0, VSLOT * s:VSLOT * s + 64] = bvv[hsl(h)]
            bvr[0, VSLOT * s + 64] = 1.0
        wvwh, wvwl = _split8(wvw * np.float32(128.0))

        wo = np.zeros((192, F), dtype=np.float32)
        wo[0:64] = WoT[hsl(F0), :]
        wo[64:128] = WoT[hsl(F1), :]
        wo[128:192] = WoT[hsl(S), :]

        idn = np.eye(128, dtype=np.float32)

        in_maps.append({
            "xh": xh, "xl": xl,
            "wqkh": wqkh, "wqkl": wqkl,
            "bqk": bqk,
            "qs": _to_bf(qs),
            "k3": _to_bf(k3),
            "v3": _to_bf(v3),
            "wvwh": wvwh, "wvwl": wvwl,
            "bv": bvr,
            "wo": _to_bf(wo),
            "idn": _to_bf(idn),
        })
    return in_maps


def _make_runner(nc):
    """Axon-path runner (built once, reused)."""
    import jax
    import jax.numpy as jnp
    import concourse.mybir as mybir
    from concourse import bass2jax
    from jax.experimental.shard_map import shard_map
    from jax.sharding import Mesh, PartitionSpec

    bass2jax.install_neuronx_cc_hook()

    partition_name = (nc.partition_id_tensor.name
                      if nc.partition_id_tensor else None)

    REPLICATED = {"xh", "xl", "idn"}
    in_names, out_names, out_avals, zero_templates = [], [], [], []
    for alloc in nc.m.functions[0].allocations:
        if not isinstance(alloc, mybir.MemoryLocationSet):
            continue
        name = alloc.memorylocations[0].name
        if alloc.kind == "ExternalInput":
            if name != partition_name:
                in_names.append(name)
        elif alloc.kind == "ExternalOutput":
            out_names.append(name)
            shape = tuple(alloc.tensor_shape)
            dtype = mybir.dt.np(alloc.dtype)
            out_avals.append(jax.core.ShapedArray(shape, dtype))
            zero_templates.append((shape, dtype))
    n_params = len(in_names)
    n_outs = len(out_avals)
    all_names = in_names + out_names
    if partition_name is not None:
        all_names = all_names + [partition_name]
    donate = tuple(range(n_params, n_params + n_outs))
    i_out = out_names.index("out")
    i_out2 = out_names.index("out2")

    devices = jax.devices()[:NCORES]
    mesh = Mesh(np.asarray(devices), ("core",))

    def _body(*args):
        operands = list(args)
        if partition_name is not None:
            operands.append(bass2jax.partition_id_tensor())
        outs = bass2jax._bass_exec_p.bind(
            *operands,
            out_avals=tuple(out_avals),
            in_names=tuple(all_names),
            out_names=tuple(out_names),
            lowering_input_output_aliases=(),
            sim_require_finite=True,
            sim_require_nnan=True,
            nc=nc,
        )
        return tuple(outs)

    in_specs = tuple(
        PartitionSpec() if n in REPLICATED else PartitionSpec("core")
        for n in in_names
    ) + (PartitionSpec("core"),) * n_outs
    bass_fn = jax.jit(
        shard_map(_body, mesh=mesh, in_specs=in_specs,
                  out_specs=(PartitionSpec("core"),) * n_outs,
                  check_rep=False),
        donate_argnums=donate, keep_unused=True,
    )

    def _zeros():
        return tuple(jnp.zeros(s, d) for (s, d) in zero_templates)

    zeros_fn = jax.jit(
        shard_map(_zeros, mesh=mesh, in_specs=(),
                  out_specs=(PartitionSpec("core"),) * n_outs,
                  check_rep=False))

    def _combine(o, o2):
        idx = jax.lax.axis_index("core")
        off = 750 * (idx % 2)
        o = o.astype(jnp.float32)
        z = jnp.zeros((T, F), jnp.float32)
        z = jax.lax.dynamic_update_slice(
            z, o2[0:750].astype(jnp.float32), (off, 0))
        return jax.lax.psum(o + z, "core")

    reduce_fn = jax.jit(
        shard_map(_combine, mesh=mesh,
                  in_specs=(PartitionSpec("core"), PartitionSpec("core")),
                  out_specs=PartitionSpec(), check_rep=False))

    dev_cache = {}

    def run(in_maps):
        args = []
        for n in in_names:
            if n in REPLICATED:
                arr = np.asarray(in_maps[0][n])
            else:
                arr = np.concatenate(
                    [np.asarray(in_maps[c][n]) for c in range(NCORES)],
                    axis=0)
            fp = (arr.shape, hash(arr.tobytes()))
            cached = dev_cache.get(n)
            if cached is not None and cached[0] == fp:
                args.append(cached[1])
            else:
                dev_arr = jax.device_put(
                    arr, jax.sharding.NamedSharding(
                        mesh,
                        PartitionSpec() if n in REPLICATED
                        else PartitionSpec("core")))
                dev_cache[n] = (fp, dev_arr)
                args.append(dev_arr)
        zeros = zeros_fn()
        outs = bass_fn(*args, *zeros)
        total = reduce_fn(outs[i_out], outs[i_out2])
        return np.asarray(total)

    return run


def kernel(x, Wq, bq, Wk, Wv, bv, Wo, bo):
    global LAST_RESULTS

    nc = _get_nc()
    in_maps = _prep_all(x, Wq, bq, Wk, Wv, bv, Wo)

    from concourse._compat import axon_active

    if axon_active():
        key = "runner"
        if key not in _CACHE:
            _CACHE[key] = _make_runner(nc)
        out = np.array(_CACHE[key](in_maps), dtype=np.float32)
    else:
        from concourse.bass_utils import run_bass_kernel_spmd
        trace = os.environ.get("KERNEL_TRACE", "0") == "1"
        res = run_bass_kernel_spmd(nc, in_maps, core_ids=list(range(NCORES)),
                                   trace=trace)
        LAST_RESULTS = res
        out = np.zeros((T, F), dtype=np.float32)
        for c in range(NCORES):
            out += np.asarray(res.results[c]["out"], dtype=np.float32)
            roff = 750 * (c % 2)
            out[roff:roff + 750] += np.asarray(res.results[c]["out2"],
                                               dtype=np.float32)
    out += np.asarray(bo, dtype=np.float32)
    return out.reshape(1, T, F)


# revision 27
# speedup vs baseline: 1.0088x; 1.0088x over previous
"""Multi-head attention (B=1, T=1500, N=1280, H=20, D=64) on 8 NeuronCores.

Tensor-parallel by head groups, 2.5 heads/core: core c owns full heads
F0=2c, F1=2c+1 plus half of shared head S=16+c//2 (q rows 750*(c%2)..+750).

Baseline bf16 dataflow (windows S, F0, F1 + drain; ACT-paced exp stream)
with the q/k/v PROJECTIONS done as fp8(e4m3) residual-3 DoubleRow
matmuls: a ~= a_h + a_l split on the host, products XhWh + XhWl + XlWh
in DoubleRow K=256 chunks at 0.5 cyc/col -> 0.75x the bf16 PE cost at
better-than-bf16 accuracy.  The F0/F1 windows are exp(ACT)-paced, so
only PE-paced phases profit from fp8: the projection front (S window)
and the drain.  x ships as xh+xl (same bytes as bf16); all input DMAs
ride one FIFO queue ordered [wqk pair, xh chunks, xl chunks, rest] so
the readiness-based scheduler cannot dilute the projection stream;
qs/k3 head the two other queues to start the S exp trickle at ~4us.

fp8 details that matter: dual-fp8 ldweights needs 64B-aligned slab
strides (x pair-chunks padded to stride 1536), and e4m3 goes subnormal
below 2^-6 (the QK_SCALE-folded k weights sit at sigma~0.004), so the
host pre-scales w by 128 and x by 8 and the psum->SBUF stage ops fold
1/1024 back in (activation Identity scale= / scalar_tensor_tensor).
m1's first chunk rides inside m0's x-chasing passes so K12 chunk 0 is
ready when the last xl chunk lands; bqk/bv head the short gpsimd DMA
queue (a late bias DMA once gated the whole F0 window).

Other structure is inherited from the bf16 predecessor:
  - software-pipelined by head in window order S, F0, F1,
  - shared head's small projections computed on the host and DMA'd in,
  - F0 hides the v-projection (2 slots x 66 cols incl. a ones column
    that makes wv emit the softmax denominator Z),
  - F1 hides wv of S and F0 (flipped [q, d] layout, deferred PE
    transposes), exp without max subtraction,
  - drain: F1's wv chain + all out-projections; bf16 output DMAs issue
    from SP; the 8 cores' partials are summed in jax.
"""

import os

import numpy as np

T, F, D = 1500, 1280, 64
NH = 20
QK_SCALE = D ** (-0.5)
NCORES = 8

PT = [128] * 11 + [92]          # partition tiles along t (sum = 1500)
PT_OFF = [128 * i for i in range(12)]
FULLSUB = [(0, 512), (512, 512), (1024, 476)]
SSUB = [(0, 512), (512, 238)]   # shared head q window = 750
SQT = 125                       # shared head q-tile width (6 x 125 = 750)
OCH = [(0, 512), (512, 512), (1024, 256)]
VSLOT = 66                      # 64 v cols + ones + pad
VW = 2 * VSLOT                  # 132 (two full-head slots)

_CACHE = {}
LAST_RESULTS = None


def _build():
    import concourse.bacc as bacc
    import concourse.mybir as mybir
    import concourse.tile as tile

    from concourse.alu_op_type import AluOpType

    f32 = mybir.dt.float32
    bf16 = mybir.dt.bfloat16
    e4 = mybir.dt.float8e4
    DR = mybir.MatmulPerfMode.DoubleRow
    IDF = mybir.ActivationFunctionType.Identity
    # fp8 pre-scales: w*128 / x*8 keep e4m3 operands out of the subnormal
    # range (QK_SCALE-folded k weights sit at sigma~0.004 unscaled); the
    # psum->SBUF stage ops fold the inverse back in.
    UNSCALE = 1.0 / 1024.0

    nc = bacc.Bacc("TRN2", target_bir_lowering=False, debug=False,
                   num_devices=NCORES)

    xh_d = nc.dram_tensor("xh", [F, T], e4, kind="ExternalInput").ap()
    xl_d = nc.dram_tensor("xl", [F, T], e4, kind="ExternalInput").ap()
    wqkh_d = nc.dram_tensor("wqkh", [F, 256], e4, kind="ExternalInput").ap()
    wqkl_d = nc.dram_tensor("wqkl", [F, 256], e4, kind="ExternalInput").ap()
    bqk_d = nc.dram_tensor("bqk", [128, 1], f32, kind="ExternalInput").ap()
    qs_d = nc.dram_tensor("qs", [64, 750], bf16, kind="ExternalInput").ap()
    k3_d = nc.dram_tensor("k3", [64, T], bf16, kind="ExternalInput").ap()
    v3_d = nc.dram_tensor("v3", [1536, VSLOT], bf16,
                          kind="ExternalInput").ap()
    wvwh_d = nc.dram_tensor("wvwh", [F, VW], e4, kind="ExternalInput").ap()
    wvwl_d = nc.dram_tensor("wvwl", [F, VW], e4, kind="ExternalInput").ap()
    bv_d = nc.dram_tensor("bv", [1, VW], f32, kind="ExternalInput").ap()
    wo_d = nc.dram_tensor("wo", [192, F], bf16, kind="ExternalInput").ap()
    idn_d = nc.dram_tensor("idn", [128, 128], bf16,
                           kind="ExternalInput").ap()
    out_d = nc.dram_tensor("out", [T, F], bf16, kind="ExternalOutput").ap()
    out2_d = nc.dram_tensor("out2", [750, F], bf16,
                            kind="ExternalOutput").ap()

    EXP = mybir.ActivationFunctionType.Exp

    import concourse.bass as bass

    with tile.TileContext(nc) as tc:
        from contextlib import ExitStack
        with ExitStack() as ctx:
            persist = ctx.enter_context(tc.tile_pool(name="persist", bufs=1))

            # ---------------- persistent SBUF ----------------
            # x pair-chunks: XH[j] [128, 3072]: cols 0:1500 = x rows
            # 256j..256j+128, cols 1536:3036 = rows 256j+128..256j+256
            # (slab stride 1536: dual-fp8 ldweights requires 64B-aligned
            # slab strides -- s3_lw_dual_fp8_restrictions)
            XSTR = 1536
            XH = [persist.tile([128, 2 * XSTR], e4, tag=f"xh{j}",
                               name=f"xh{j}") for j in range(5)]
            XL = [persist.tile([128, 2 * XSTR], e4, tag=f"xl{j}",
                               name=f"xl{j}") for j in range(5)]
            WQKH = persist.tile([128, 2560], e4, tag="wqkh", name="wqkh")
            WQKL = persist.tile([128, 2560], e4, tag="wqkl", name="wqkl")
            WVWH = persist.tile([128, 10 * VW], e4, tag="wvwh", name="wvwh")
            WVWL = persist.tile([128, 10 * VW], e4, tag="wvwl", name="wvwl")
            Q12 = persist.tile([128, T], bf16, tag="q12", name="q12")
            K12 = persist.tile([128, T], bf16, tag="k12", name="k12")
            K3 = persist.tile([64, T], bf16, tag="k3", name="k3")
            QS = persist.tile([64, 750], bf16, tag="qs", name="qs")
            V = persist.tile([128, 12 * VW], bf16, tag="vall", name="vall")
            V3 = persist.tile([128, 12 * VSLOT], bf16, tag="v3", name="v3")
            WVT12 = persist.tile([128, T], bf16, tag="wvt12", name="wvt12")
            WVT3 = persist.tile([64, 750], bf16, tag="wvt3", name="wvt3")
            WO12 = persist.tile([128, F], bf16, tag="wo12", name="wo12")
            WO3 = persist.tile([64, F], bf16, tag="wo3", name="wo3")
            BQK = persist.tile([128, 1], f32, tag="bqk", name="bqk")
            BV = persist.tile([128, VW], f32, tag="bv", name="bv")
            IDN = persist.tile([128, 128], bf16, tag="idn", name="idn")

            epool = ctx.enter_context(tc.tile_pool(name="epool", bufs=24))
            fin = ctx.enter_context(tc.tile_pool(name="fin", bufs=12))
            ost = ctx.enter_context(tc.tile_pool(name="ost", bufs=18))

            # ---------------- input DMAs ----------------
            # Transfers round-robin across the HWDGE queues by READINESS
            # (not emission), so qs/k3 head the two short queues (the S
            # trickle starts ~4us) and every other input rides the sync
            # FIFO in priority order: the projection stream (wqk pair,
            # xh chunks, xl chunks) cannot be diluted by later tensors.
            def batched_chunks(dst_tile, src, width, f0=0, nf=10,
                               eng=None):
                d0 = dst_tile[:, :]
                dst = bass.AP(tensor=d0.tensor,
                              offset=d0.offset + f0 * width,
                              ap=[list(d0.ap)[0], [width, nf], [1, width]])
                s0 = src[0:128, :]
                dsrc = bass.AP(tensor=s0.tensor,
                               offset=s0.offset + f0 * 128 * width,
                               ap=[list(s0.ap)[0], [128 * width, nf],
                                   [1, width]])
                (eng or nc.sync).dma_start(dst, dsrc)

            nc.gpsimd.dma_start(QS[:], qs_d[:, :])
            nc.gpsimd.dma_start(BQK[:], bqk_d[:, :])
            bvs = bv_d[0:1, :]
            bv_bc = bass.AP(tensor=bvs.tensor, offset=bvs.offset,
                            ap=[[0, 128]] + list(bvs.ap)[1:])
            nc.gpsimd.dma_start(BV[:], bv_bc)
            # k3 split: the first S score tile needs only cols 0:128, so a
            # small head transfer starts the exp trickle earlier
            nc.scalar.dma_start(K3[0:64, 0:256], k3_d[:, 0:256])
            nc.scalar.dma_start(K3[0:64, 256:T], k3_d[:, 256:T])
            batched_chunks(WQKH, wqkh_d, 256)
            batched_chunks(WQKL, wqkl_d, 256)

            def x_pair_dma(dst_tile, src, j):
                d0 = dst_tile[:, :]
                dst = bass.AP(tensor=d0.tensor, offset=d0.offset,
                              ap=[list(d0.ap)[0], [XSTR, 2], [1, T]])
                s0 = src[0:128, :]
                dsrc = bass.AP(tensor=s0.tensor,
                               offset=s0.offset + 256 * j * T,
                               ap=[list(s0.ap)[0], [128 * T, 2], [1, T]])
                nc.sync.dma_start(dst, dsrc)

            for j in range(5):
                x_pair_dma(XH[j], xh_d, j)
            for j in range(5):
                x_pair_dma(XL[j], xl_d, j)
            batched_chunks(WVWH, wvwh_d, VW)
            batched_chunks(WVWL, wvwl_d, VW)
            v3dst = V3[:, :]
            v3src = bass.AP(tensor=v3_d.tensor, offset=0,
                            ap=[[VSLOT, 128], [128 * VSLOT, 12], [1, VSLOT]])
            v3dst3 = bass.AP(tensor=v3dst.tensor, offset=v3dst.offset,
                             ap=[list(v3dst.ap)[0], [VSLOT, 12], [1, VSLOT]])
            nc.sync.dma_start(v3dst3, v3src)
            nc.sync.dma_start(IDN[:], idn_d[:, :])
            nc.sync.dma_start(WO12[:], wo_d[0:128, :])
            nc.sync.dma_start(WO3[:], wo_d[128:192, :])

            def ap3(tile_ap, counts_strides, off=0):
                return bass.AP(tensor=tile_ap.tensor,
                               offset=tile_ap.offset + off,
                               ap=[list(tile_ap.ap)[0]] +
                                  [[s, c] for (s, c) in counts_strides])

            # ---------------- pipelined attention ----------------
            att_ctx = ExitStack()
            sc_ctx = ExitStack()
            sc_pool = [sc_ctx.enter_context(
                tc.tile_pool(name="pp_scs", bufs=2, space="PSUM"))]
            qk_ctx = ExitStack()
            pp_qk = qk_ctx.enter_context(
                tc.tile_pool(name="pp_qk", bufs=4, space="PSUM",
                             side="right"))

            E12 = [[None] * 12 for _ in range(3)]
            pend = {}   # wv tiles awaiting their deferred transpose
            cp_rr = [0]
            att_pool = [None]
            wv_bufs = [1]
            pp_o = None

            def emit_scores(h, kt):
                pk = PT[kt]
                wq = T if h < 2 else 750
                ps = sc_pool[0].tile([128, wq], f32, tag="psc", name="psc")
                sub = FULLSUB if h < 2 else SSUB
                if h < 2:
                    lh = K12[64 * h:64 * h + 64, PT_OFF[kt]:PT_OFF[kt] + pk]
                    qsrc = Q12[64 * h:64 * h + 64, :]
                else:
                    lh = K3[0:64, PT_OFF[kt]:PT_OFF[kt] + pk]
                    qsrc = QS[0:64, :]
                for (o, w) in sub:
                    nc.tensor.matmul(
                        ps[0:pk, o:o + w], lh, qsrc[:, o:o + w],
                        start=True, stop=True,
                    )
                if h < 2:
                    E = epool.tile([128, T], bf16, tag="E", name="E")
                else:
                    E = epool.tile([128, 750], bf16, tag="ES", bufs=12,
                                   name="ES")
                nc.scalar.activation(E[0:pk, 0:wq], ps[0:pk, 0:wq], EXP)
                E12[h][kt] = E

            PRODS = ((XH, 0), (XH, 1), (XL, 0))  # (x level, w level)

            def emit_vproj(tt):
                pk = PT[tt]
                ps = pp_v.tile([128, VW], f32, tag="pv", name="pv")
                n = 0
                for (XS, wl) in PRODS:
                    WT = WVWH if wl == 0 else WVWL
                    for j in range(5):
                        lhsT = ap3(XS[j][:, :], [(XSTR, 2), (1, pk)],
                                   off=PT_OFF[tt])
                        rhs = ap3(WT[:, :], [(VW, 2), (1, VW)],
                                  off=2 * VW * j)
                        nc.tensor.matmul(
                            ps[0:pk, :], lhsT, rhs,
                            start=(n == 0), stop=(n == 14), perf_mode=DR,
                        )
                        n += 1
                nc.vector.scalar_tensor_tensor(
                    V[0:pk, VW * tt:VW * (tt + 1)], ps[0:pk, :], UNSCALE,
                    BV[0:pk, :], AluOpType.mult, AluOpType.add)

            def emit_wv(h, qt):
                """wv for head h, q-tile qt (kt-inner accumulation) plus the
                DVE normalize; the PE transpose is deferred to flush_norm."""
                if h < 2:
                    pkq, qo = PT[qt], PT_OFF[qt]
                else:
                    pkq, qo = SQT, SQT * qt
                ps = att_pool[0].tile([128, VSLOT], f32, tag="wv",
                                      bufs=wv_bufs[0], name="wv")
                for kt in range(12):
                    pkk = PT[kt]
                    if h < 2:
                        vsrc = V[0:pkk, VW * kt + VSLOT * h:
                                 VW * kt + VSLOT * h + VSLOT]
                    else:
                        vsrc = V3[0:pkk, VSLOT * kt:VSLOT * (kt + 1)]
                    nc.tensor.matmul(
                        ps[0:pkq, 0:VSLOT],
                        E12[h][kt][0:pkk, qo:qo + pkq],
                        vsrc,
                        start=(kt == 0), stop=(kt == 11),
                    )
                rz = fin.tile([128, 1], f32, tag="rz", name="rz")
                nc.vector.reciprocal(rz[0:pkq, 0:1], ps[0:pkq, 64:65])
                wsb = fin.tile([128, 64], bf16, tag="wsb", name="wsb")
                nc.vector.tensor_scalar_mul(wsb[0:pkq, 0:64],
                                            ps[0:pkq, 0:64], rz[0:pkq, 0:1])
                pend[(h, qt)] = (wsb, pkq, qo)

            def flush_norm(h, qt):
                """PE transpose + WVT copy for a pending wv tile."""
                wsb, pkq, qo = pend.pop((h, qt))
                pst = att_pool[0].tile([64, 128], bf16, tag="pt", name="pt")
                nc.tensor.transpose(pst[0:64, 0:pkq], wsb[0:pkq, 0:64],
                                    IDN[0:pkq, 0:pkq])
                # h=0/2 run inside exp windows: keep ACT free for exp.
                # h=1 runs in the drain where ACT has slack.
                cp = nc.scalar.copy if h == 1 else nc.vector.tensor_copy
                if h < 2:
                    cp(WVT12[64 * h:64 * h + 64, qo:qo + pkq],
                       pst[0:64, 0:pkq])
                else:
                    cp(WVT3[0:64, qo:qo + pkq], pst[0:64, 0:pkq])

            def emit_outproj(lhs, wo, dst, pk, dsl):
                stage = ost.tile([128, F], bf16, tag="stage", name="stage")
                ps = pp_o.tile([128, 1024], f32, tag="po", name="po")
                pst2 = att_pool[0].tile([128, 256], f32, tag="po256",
                                        name="po256")
                for (o, w) in ((0, 512), (512, 512)):
                    nc.tensor.matmul(
                        ps[0:pk, o:o + w], lhs, wo[:, o:o + w],
                        start=True, stop=True,
                    )
                nc.tensor.matmul(pst2[0:pk, 0:256], lhs, wo[:, 1024:1280],
                                 start=True, stop=True)
                cp_rr[0] += 1
                a, b = ((nc.scalar.copy, nc.vector.tensor_copy)
                        if cp_rr[0] % 2 else
                        (nc.vector.tensor_copy, nc.scalar.copy))
                a(stage[0:pk, 0:1024], ps[0:pk, 0:1024])
                b(stage[0:pk, 1024:1280], pst2[0:pk, 0:256])
                nc.sync.dma_start(dst[dsl, :], stage[0:pk, :])

            def emit_outproj12(tt):
                pk = PT[tt]
                tsl = slice(PT_OFF[tt], PT_OFF[tt] + pk)
                emit_outproj(WVT12[:, tsl], WO12, out_d, pk, tsl)

            def emit_outproj3(qt):
                qsl = slice(SQT * qt, SQT * (qt + 1))
                emit_outproj(WVT3[:, qsl], WO3, out2_d, SQT, qsl)

            # ---- window S: shared-head scores/exp, projection fillers ----
            for kt in range(12):
                emit_scores(2, kt)

            # m0 (q) + m1's FIRST chunk: res-3 DoubleRow in two passes
            # matching the x DMA order (Xh products chase the xh chunks,
            # then the Xl product chases xl).  Riding m1-c0 inside the
            # x-chase means k chunk 0 (and F0's first score tiles) is
            # ready right when the last xl chunk lands instead of two
            # chunk-times later.
            ps0 = [pp_qk.tile([128, 512], f32, tag="pqk", name="pqk",
                              bufs=4) for _ in range(3)]
            ps1c0 = pp_qk.tile([128, 512], f32, tag="pqk", name="pqk",
                               bufs=4)
            n = 0
            for pr in (((XH, 0), (XH, 1)), ((XL, 0),)):
                for j in range(5):
                    for (XS, wl) in pr:
                        WT = WQKH if wl == 0 else WQKL
                        lhsT = ap3(WT[:, :], [(256, 2), (1, 128)],
                                   off=512 * j)
                        for ci, (o, w) in enumerate(FULLSUB):
                            rhs = ap3(XS[j][:, :], [(XSTR, 2), (1, w)],
                                      off=o)
                            nc.tensor.matmul(ps0[ci][0:128, 0:w], lhsT, rhs,
                                             start=(n == 0), stop=(n == 14),
                                             perf_mode=DR)
                        lhsTk = ap3(WT[:, :], [(256, 2), (1, 128)],
                                    off=512 * j + 128)
                        rhsk = ap3(XS[j][:, :], [(XSTR, 2), (1, 512)],
                                   off=0)
                        nc.tensor.matmul(ps1c0[0:128, 0:512], lhsTk, rhsk,
                                         start=(n == 0), stop=(n == 14),
                                         perf_mode=DR)
                        n += 1
            nc.scalar.activation(K12[:, 0:512], ps1c0[0:128, 0:512],
                                 IDF, scale=UNSCALE)
            for ci, (o, w) in enumerate(FULLSUB):
                nc.scalar.activation(Q12[:, o:o + w], ps0[ci][0:128, 0:w],
                                     IDF, bias=BQK[:, 0:1], scale=UNSCALE)

            # m1 (k) remaining chunks: C-MAJOR res-3 with an inline copy
            # per chunk
            for ci, (o, w) in list(enumerate(FULLSUB))[1:]:
                ps1 = pp_qk.tile([128, 512], f32, tag="pqk", name="pqk",
                                 bufs=4)
                n = 0
                for (XS, wl) in PRODS:
                    WT = WQKH if wl == 0 else WQKL
                    for j in range(5):
                        lhsT = ap3(WT[:, :], [(256, 2), (1, 128)],
                                   off=512 * j + 128)
                        rhs = ap3(XS[j][:, :], [(XSTR, 2), (1, w)],
                                  off=o)
                        nc.tensor.matmul(ps1[0:128, 0:w], lhsT, rhs,
                                         start=(n == 0), stop=(n == 14),
                                         perf_mode=DR)
                        n += 1
                # k has no bias; the copy folds the fp8 pre-scale away
                nc.scalar.activation(K12[:, o:o + w], ps1[0:128, 0:w],
                                     IDF, scale=UNSCALE)
            qk_ctx.close()

            # ---- window F0: scores/exp + v-proj fillers ----
            sc_ctx.close()
            sc_ctx = ExitStack()
            sc_pool[0] = sc_ctx.enter_context(
                tc.tile_pool(name="pp_sc", bufs=2, space="PSUM"))
            vp_ctx = ExitStack()
            pp_v = vp_ctx.enter_context(
                tc.tile_pool(name="pp_v", bufs=2, space="PSUM",
                             side="right"))
            for kt in range(12):
                emit_scores(0, kt)
                emit_vproj(kt)
            vp_ctx.close()

            # ---- window F1: scores/exp + F0 wv + shared-head wv ----
            att_pool[0] = att_ctx.enter_context(
                tc.tile_pool(name="pp_att", bufs=1, space="PSUM",
                             side="right"))
            for kt in range(12):
                emit_scores(1, kt)
                emit_wv(0, kt)
                if kt % 2 == 1:
                    emit_wv(2, kt // 2)
                if kt >= 1:
                    flush_norm(0, kt - 1)
                if kt % 2 == 0 and kt >= 2:
                    flush_norm(2, kt // 2 - 1)
            flush_norm(0, 11)
            flush_norm(2, 5)

            # ---- drain: F1 wv + all out-projections ----
            sc_ctx.close()
            att_ctx.close()
            att_ctx = ExitStack()
            att_pool[0] = att_ctx.enter_context(
                tc.tile_pool(name="pp_att2", bufs=1, space="PSUM",
                             side="right"))
            wv_bufs[0] = 2
            o_ctx = ExitStack()
            pp_o = o_ctx.enter_context(
                tc.tile_pool(name="pp_o", bufs=2, space="PSUM"))
            # out2's projections first: WVT3 is complete at F1 end, so
            # their stage copies give ACT/DVE work at drain start
            for qt in range(6):
                emit_outproj3(qt)
            for qt in range(12):
                emit_wv(1, qt)
                if qt >= 1:
                    flush_norm(1, qt - 1)
            flush_norm(1, 11)
            for tt in range(12):
                emit_outproj12(tt)
            o_ctx.close()
            att_ctx.close()

    nc.compile()
    return nc


def _get_nc(_unused=None):
    if "nc" not in _CACHE:
        _CACHE["nc"] = _build()
    return _CACHE["nc"]


def _split8(a):
    import ml_dtypes
    a = np.asarray(a, dtype=np.float32)
    hi = a.astype(ml_dtypes.float8_e4m3)
    lo = (a - hi.astype(np.float32)).astype(ml_dtypes.float8_e4m3)
    return hi, lo


def _to_bf(a):
    import ml_dtypes
    return np.ascontiguousarray(np.asarray(a, dtype=np.float32)).astype(
        ml_dtypes.bfloat16)


def _prep_all(x, Wq, bq, Wk, Wv, bv, Wo):
    x = np.asarray(x, dtype=np.float32).reshape(T, F)
    xT = np.ascontiguousarray(x.T)
    xh, xl = _split8(xT * np.float32(8.0))
    WqT = np.ascontiguousarray(np.asarray(Wq, dtype=np.float32).T)
    WkTs = (np.ascontiguousarray(np.asarray(Wk, dtype=np.float32).T)
            * np.float32(QK_SCALE))
    WvT = np.ascontiguousarray(np.asarray(Wv, dtype=np.float32).T)
    WoT = np.ascontiguousarray(np.asarray(Wo, dtype=np.float32).T)
    bq = np.asarray(bq, dtype=np.float32)
    bvv = np.asarray(bv, dtype=np.float32)

    def hsl(h):
        return slice(D * h, D * (h + 1))

    in_maps = []
    for c in range(NCORES):
        F0, F1 = 2 * c, 2 * c + 1
        S = 16 + c // 2

        wqk = np.zeros((F, 256), dtype=np.float32)
        wqk[:, 0:64] = WqT[:, hsl(F0)]
        wqk[:, 64:128] = WqT[:, hsl(F1)]
        wqk[:, 128:192] = WkTs[:, hsl(F0)]
        wqk[:, 192:256] = WkTs[:, hsl(F1)]
        wqkh, wqkl = _split8(wqk * np.float32(128.0))

        bqk = np.zeros((128, 1), dtype=np.float32)
        bqk[0:64, 0] = bq[hsl(F0)]
        bqk[64:128, 0] = bq[hsl(F1)]

        roff = 750 * (c % 2)
        qs = (x[roff:roff + 750] @ WqT[:, hsl(S)] + bq[hsl(S)]).T
        k3 = (x @ WkTs[:, hsl(S)]).T
        v3 = np.zeros((1536, VSLOT), dtype=np.float32)
        v3[0:T, 0:64] = x @ WvT[:, hsl(S)] + bvv[hsl(S)]
        v3[0:T, 64] = 1.0

        wvw = np.zeros((F, VW), dtype=np.float32)
        bvr = np.zeros((1, VW), dtype=np.float32)
        for s, h in enumerate((F0, F1)):
            wvw[:, VSLOT * s:VSLOT * s + 64] = WvT[:, hsl(h)]
            bvr[0, VSLOT * s:VSLOT * s + 64] = bvv[hsl(h)]
            bvr[0, VSLOT * s + 64] = 1.0
        wvwh, wvwl = _split8(wvw * np.float32(128.0))

        wo = np.zeros((192, F), dtype=np.float32)
        wo[0:64] = WoT[hsl(F0), :]
        wo[64:128] = WoT[hsl(F1), :]
        wo[128:192] = WoT[hsl(S), :]

        idn = np.eye(128, dtype=np.float32)

        in_maps.append({
            "xh": xh, "xl": xl,
            "wqkh": wqkh, "wqkl": wqkl,
            "bqk": bqk,
            "qs": _to_bf(qs),
            "k3": _to_bf(k3),
            "v3": _to_bf(v3),
            "wvwh": wvwh, "wvwl": wvwl,
            "bv": bvr,
            "wo": _to_bf(wo),
            "idn": _to_bf(idn),
        })
    return in_maps


def _make_runner(nc):
    """Axon-path runner (built once, reused)."""
    import jax
    import jax.numpy as jnp
    import concourse.mybir as mybir
    from concourse import bass2jax
    from jax.experimental.shard_map import shard_map
    from jax.sharding import Mesh, PartitionSpec

    bass2jax.install_neuronx_cc_hook()

    partition_name = (nc.partition_id_tensor.name
                      if nc.partition_id_tensor else None)

    REPLICATED = {"xh", "xl", "idn"}
    in_names, out_names, out_avals, zero_templates = [], [], [], []
    for alloc in nc.m.functions[0].allocations:
        if not isinstance(alloc, mybir.MemoryLocationSet):
            continue
        name = alloc.memorylocations[0].name
        if alloc.kind == "ExternalInput":
            if name != partition_name:
                in_names.append(name)
        elif alloc.kind == "ExternalOutput":
            out_names.append(name)
            shape = tuple(alloc.tensor_shape)
            dtype = mybir.dt.np(alloc.dtype)
            out_avals.append(jax.core.ShapedArray(shape, dtype))
            zero_templates.append((shape, dtype))
    n_params = len(in_names)
    n_outs = len(out_avals)
    all_names = in_names + out_names
    if partition_name is not None:
        all_names = all_names + [partition_name]
    donate = tuple(range(n_params, n_params + n_outs))
    i_out = out_names.index("out")
    i_out2 = out_names.index("out2")

    devices = jax.devices()[:NCORES]
    mesh = Mesh(np.asarray(devices), ("core",))

    def _body(*args):
        operands = list(args)
        if partition_name is not None:
            operands.append(bass2jax.partition_id_tensor())
        outs = bass2jax._bass_exec_p.bind(
            *operands,
            out_avals=tuple(out_avals),
            in_names=tuple(all_names),
            out_names=tuple(out_names),
            lowering_input_output_aliases=(),
            sim_require_finite=True,
            sim_require_nnan=True,
            nc=nc,
        )
        return tuple(outs)

    in_specs = tuple(
        PartitionSpec() if n in REPLICATED else PartitionSpec("core")
        for n in in_names
    ) + (PartitionSpec("core"),) * n_outs
    bass_fn = jax.jit(
        shard_map(_body, mesh=mesh, in_specs=in_specs,
                  out_specs=(PartitionSpec("core"),) * n_outs,
                  check_rep=False),
        donate_argnums=donate, keep_unused=True,
    )

    def _zeros():
        return tuple(jnp.zeros(s, d) for (s, d) in zero_templates)

    zeros_fn = jax.jit(
        shard_map(_zeros, mesh=mesh, in_specs=(),
                  out_specs=(PartitionSpec("core"),) * n_outs,
                  check_rep=False))

    def _combine(o, o2):
        idx = jax.lax.axis_index("core")
        off = 750 * (idx % 2)
        o = o.astype(jnp.float32)
        z = jnp.zeros((T, F), jnp.float32)
        z = jax.lax.dynamic_update_slice(
            z, o2[0:750].astype(jnp.float32), (off, 0))
        return jax.lax.psum(o + z, "core")

    reduce_fn = jax.jit(
        shard_map(_combine, mesh=mesh,
                  in_specs=(PartitionSpec("core"), PartitionSpec("core")),
                  out_specs=PartitionSpec(), check_rep=False))

    dev_cache = {}

    def run(in_maps):
        args = []
        for n in in_names:
            if n in REPLICATED:
                arr = np.asarray(in_maps[0][n])
            else:
                arr = np.concatenate(
                    [np.asarray(in_maps[c][n]) for c in range(NCORES)],
                    axis=0)
            fp = (arr.shape, hash(arr.tobytes()))
            cached = dev_cache.get(n)
            if cached is not None and cached[0] == fp:
                args.append(cached[1])
            else:
                dev_arr = jax.device_put(
                    arr, jax.sharding.NamedSharding(
                        mesh,
                        PartitionSpec() if n in REPLICATED
                        else PartitionSpec("core")))
                dev_cache[n] = (fp, dev_arr)
                args.append(dev_arr)
        zeros = zeros_fn()
        outs = bass_fn(*args, *zeros)
        total = reduce_fn(outs[i_out], outs[i_out2])
        return np.asarray(total)

    return run


def kernel(x, Wq, bq, Wk, Wv, bv, Wo, bo):
    global LAST_RESULTS

    nc = _get_nc()
    in_maps = _prep_all(x, Wq, bq, Wk, Wv, bv, Wo)

    from concourse._compat import axon_active

    if axon_active():
        key = "runner"
        if key not in _CACHE:
            _CACHE[key] = _make_runner(nc)
        out = np.array(_CACHE[key](in_maps), dtype=np.float32)
    else:
        from concourse.bass_utils import run_bass_kernel_spmd
        trace = os.environ.get("KERNEL_TRACE", "0") == "1"
        res = run_bass_kernel_spmd(nc, in_maps, core_ids=list(range(NCORES)),
                                   trace=trace)
        LAST_RESULTS = res
        out = np.zeros((T, F), dtype=np.float32)
        for c in range(NCORES):
            out += np.asarray(res.results[c]["out"], dtype=np.float32)
            roff = 750 * (c % 2)
            out[roff:roff + 750] += np.asarray(res.results[c]["out2"],
                                               dtype=np.float32)
    out += np.asarray(bo, dtype=np.float32)
    return out.reshape(1, T, F)


# revision 28
# speedup vs baseline: 1.0097x; 1.0009x over previous
"""Multi-head attention (B=1, T=1500, N=1280, H=20, D=64) on 8 NeuronCores.

Tensor-parallel by head groups, 2.5 heads/core: core c owns full heads
F0=2c, F1=2c+1 plus half of shared head S=16+c//2 (q rows 750*(c%2)..+750).

Baseline bf16 dataflow (windows S, F0, F1 + drain; ACT-paced exp stream)
with the q/k/v PROJECTIONS done as fp8(e4m3) residual-3 DoubleRow
matmuls: a ~= a_h + a_l split on the host, products XhWh + XhWl + XlWh
in DoubleRow K=256 chunks at 0.5 cyc/col -> 0.75x the bf16 PE cost at
better-than-bf16 accuracy.  The F0/F1 windows are exp(ACT)-paced, so
only PE-paced phases profit from fp8: the projection front (S window)
and the drain.  x ships as xh+xl (same bytes as bf16); all input DMAs
ride one FIFO queue ordered [wqk pair, xh chunks, xl chunks, rest] so
the readiness-based scheduler cannot dilute the projection stream;
qs/k3 head the two other queues to start the S exp trickle at ~4us.

fp8 details that matter: dual-fp8 ldweights needs 64B-aligned slab
strides (x pair-chunks padded to stride 1536), and e4m3 goes subnormal
below 2^-6 (the QK_SCALE-folded k weights sit at sigma~0.004), so the
host pre-scales w by 128 and x by 8 and the psum->SBUF stage ops fold
1/1024 back in (activation Identity scale= / scalar_tensor_tensor).
m1's first chunk rides inside m0's x-chasing passes so K12 chunk 0 is
ready when the last xl chunk lands; bqk/bv head the short gpsimd DMA
queue (a late bias DMA once gated the whole F0 window).

Other structure is inherited from the bf16 predecessor:
  - software-pipelined by head in window order S, F0, F1,
  - shared head's small projections computed on the host and DMA'd in,
  - F0 hides the v-projection (2 slots x 66 cols incl. a ones column
    that makes wv emit the softmax denominator Z),
  - F1 hides wv of S and F0 (flipped [q, d] layout, deferred PE
    transposes), exp without max subtraction,
  - drain: F1's wv chain + all out-projections; bf16 output DMAs issue
    from SP; the 8 cores' partials are summed in jax.
"""

import os

import numpy as np

T, F, D = 1500, 1280, 64
NH = 20
QK_SCALE = D ** (-0.5)
NCORES = 8

PT = [128] * 11 + [92]          # partition tiles along t (sum = 1500)
PT_OFF = [128 * i for i in range(12)]
FULLSUB = [(0, 512), (512, 512), (1024, 476)]
SSUB = [(0, 512), (512, 238)]   # shared head q window = 750
SQT = 125                       # shared head q-tile width (6 x 125 = 750)
OCH = [(0, 512), (512, 512), (1024, 256)]
VSLOT = 66                      # 64 v cols + ones + pad
VW = 2 * VSLOT                  # 132 (two full-head slots)

_CACHE = {}
LAST_RESULTS = None


def _build():
    import concourse.bacc as bacc
    import concourse.mybir as mybir
    import concourse.tile as tile

    from concourse.alu_op_type import AluOpType

    f32 = mybir.dt.float32
    bf16 = mybir.dt.bfloat16
    e4 = mybir.dt.float8e4
    DR = mybir.MatmulPerfMode.DoubleRow
    IDF = mybir.ActivationFunctionType.Identity
    # fp8 pre-scales: w*128 / x*8 keep e4m3 operands out of the subnormal
    # range (QK_SCALE-folded k weights sit at sigma~0.004 unscaled); the
    # psum->SBUF stage ops fold the inverse back in.
    UNSCALE = 1.0 / 1024.0

    nc = bacc.Bacc("TRN2", target_bir_lowering=False, debug=False,
                   num_devices=NCORES)

    xh_d = nc.dram_tensor("xh", [F, T], e4, kind="ExternalInput").ap()
    xl_d = nc.dram_tensor("xl", [F, T], e4, kind="ExternalInput").ap()
    wqkh_d = nc.dram_tensor("wqkh", [F, 256], e4, kind="ExternalInput").ap()
    wqkl_d = nc.dram_tensor("wqkl", [F, 256], e4, kind="ExternalInput").ap()
    bqk_d = nc.dram_tensor("bqk", [128, 1], f32, kind="ExternalInput").ap()
    qs_d = nc.dram_tensor("qs", [64, 750], bf16, kind="ExternalInput").ap()
    k3_d = nc.dram_tensor("k3", [64, T], bf16, kind="ExternalInput").ap()
    v3_d = nc.dram_tensor("v3", [1536, VSLOT], bf16,
                          kind="ExternalInput").ap()
    wvwh_d = nc.dram_tensor("wvwh", [F, VW], e4, kind="ExternalInput").ap()
    wvwl_d = nc.dram_tensor("wvwl", [F, VW], e4, kind="ExternalInput").ap()
    bv_d = nc.dram_tensor("bv", [1, VW], f32, kind="ExternalInput").ap()
    wo_d = nc.dram_tensor("wo", [192, F], bf16, kind="ExternalInput").ap()
    idn_d = nc.dram_tensor("idn", [128, 128], bf16,
                           kind="ExternalInput").ap()
    out_d = nc.dram_tensor("out", [T, F], bf16, kind="ExternalOutput").ap()
    out2_d = nc.dram_tensor("out2", [750, F], bf16,
                            kind="ExternalOutput").ap()

    EXP = mybir.ActivationFunctionType.Exp

    import concourse.bass as bass

    with tile.TileContext(nc) as tc:
        from contextlib import ExitStack
        with ExitStack() as ctx:
            persist = ctx.enter_context(tc.tile_pool(name="persist", bufs=1))

            # ---------------- persistent SBUF ----------------
            # x pair-chunks: XH[j] [128, 3072]: cols 0:1500 = x rows
            # 256j..256j+128, cols 1536:3036 = rows 256j+128..256j+256
            # (slab stride 1536: dual-fp8 ldweights requires 64B-aligned
            # slab strides -- s3_lw_dual_fp8_restrictions)
            XSTR = 1536
            XH = [persist.tile([128, 2 * XSTR], e4, tag=f"xh{j}",
                               name=f"xh{j}") for j in range(5)]
            XL = [persist.tile([128, 2 * XSTR], e4, tag=f"xl{j}",
                               name=f"xl{j}") for j in range(5)]
            WQKH = persist.tile([128, 2560], e4, tag="wqkh", name="wqkh")
            WQKL = persist.tile([128, 2560], e4, tag="wqkl", name="wqkl")
            WVWH = persist.tile([128, 10 * VW], e4, tag="wvwh", name="wvwh")
            WVWL = persist.tile([128, 10 * VW], e4, tag="wvwl", name="wvwl")
            Q12 = persist.tile([128, T], bf16, tag="q12", name="q12")
            K12 = persist.tile([128, T], bf16, tag="k12", name="k12")
            K3 = persist.tile([64, T], bf16, tag="k3", name="k3")
            QS = persist.tile([64, 750], bf16, tag="qs", name="qs")
            V = persist.tile([128, 12 * VW], bf16, tag="vall", name="vall")
            V3 = persist.tile([128, 12 * VSLOT], bf16, tag="v3", name="v3")
            WVT12 = persist.tile([128, T], bf16, tag="wvt12", name="wvt12")
            WVT3 = persist.tile([64, 750], bf16, tag="wvt3", name="wvt3")
            WO12 = persist.tile([128, F], bf16, tag="wo12", name="wo12")
            WO3 = persist.tile([64, F], bf16, tag="wo3", name="wo3")
            BQK = persist.tile([128, 1], f32, tag="bqk", name="bqk")
            BV = persist.tile([128, VW], f32, tag="bv", name="bv")
            IDN = persist.tile([128, 128], bf16, tag="idn", name="idn")

            epool = ctx.enter_context(tc.tile_pool(name="epool", bufs=24))
            fin = ctx.enter_context(tc.tile_pool(name="fin", bufs=16))
            ost = ctx.enter_context(tc.tile_pool(name="ost", bufs=18))

            # ---------------- input DMAs ----------------
            # Transfers round-robin across the HWDGE queues by READINESS
            # (not emission), so qs/k3 head the two short queues (the S
            # trickle starts ~4us) and every other input rides the sync
            # FIFO in priority order: the projection stream (wqk pair,
            # xh chunks, xl chunks) cannot be diluted by later tensors.
            def batched_chunks(dst_tile, src, width, f0=0, nf=10,
                               eng=None):
                d0 = dst_tile[:, :]
                dst = bass.AP(tensor=d0.tensor,
                              offset=d0.offset + f0 * width,
                              ap=[list(d0.ap)[0], [width, nf], [1, width]])
                s0 = src[0:128, :]
                dsrc = bass.AP(tensor=s0.tensor,
                               offset=s0.offset + f0 * 128 * width,
                               ap=[list(s0.ap)[0], [128 * width, nf],
                                   [1, width]])
                (eng or nc.sync).dma_start(dst, dsrc)

            nc.gpsimd.dma_start(QS[:], qs_d[:, :])
            nc.gpsimd.dma_start(BQK[:], bqk_d[:, :])
            bvs = bv_d[0:1, :]
            bv_bc = bass.AP(tensor=bvs.tensor, offset=bvs.offset,
                            ap=[[0, 128]] + list(bvs.ap)[1:])
            nc.gpsimd.dma_start(BV[:], bv_bc)
            # k3 split: the first S score tile needs only cols 0:128, so a
            # small head transfer starts the exp trickle earlier
            nc.scalar.dma_start(K3[0:64, 0:256], k3_d[:, 0:256])
            nc.scalar.dma_start(K3[0:64, 256:T], k3_d[:, 256:T])
            batched_chunks(WQKH, wqkh_d, 256)
            batched_chunks(WQKL, wqkl_d, 256)

            def x_pair_dma(dst_tile, src, j):
                d0 = dst_tile[:, :]
                dst = bass.AP(tensor=d0.tensor, offset=d0.offset,
                              ap=[list(d0.ap)[0], [XSTR, 2], [1, T]])
                s0 = src[0:128, :]
                dsrc = bass.AP(tensor=s0.tensor,
                               offset=s0.offset + 256 * j * T,
                               ap=[list(s0.ap)[0], [128 * T, 2], [1, T]])
                nc.sync.dma_start(dst, dsrc)

            for j in range(5):
                x_pair_dma(XH[j], xh_d, j)
            for j in range(5):
                x_pair_dma(XL[j], xl_d, j)
            batched_chunks(WVWH, wvwh_d, VW)
            batched_chunks(WVWL, wvwl_d, VW)
            v3dst = V3[:, :]
            v3src = bass.AP(tensor=v3_d.tensor, offset=0,
                            ap=[[VSLOT, 128], [128 * VSLOT, 12], [1, VSLOT]])
            v3dst3 = bass.AP(tensor=v3dst.tensor, offset=v3dst.offset,
                             ap=[list(v3dst.ap)[0], [VSLOT, 12], [1, VSLOT]])
            nc.sync.dma_start(v3dst3, v3src)
            nc.sync.dma_start(IDN[:], idn_d[:, :])
            nc.sync.dma_start(WO12[:], wo_d[0:128, :])
            nc.sync.dma_start(WO3[:], wo_d[128:192, :])

            def ap3(tile_ap, counts_strides, off=0):
                return bass.AP(tensor=tile_ap.tensor,
                               offset=tile_ap.offset + off,
                               ap=[list(tile_ap.ap)[0]] +
                                  [[s, c] for (s, c) in counts_strides])

            # ---------------- pipelined attention ----------------
            att_ctx = ExitStack()
            sc_ctx = ExitStack()
            sc_pool = [sc_ctx.enter_context(
                tc.tile_pool(name="pp_scs", bufs=2, space="PSUM"))]
            qk_ctx = ExitStack()
            pp_qk = qk_ctx.enter_context(
                tc.tile_pool(name="pp_qk", bufs=4, space="PSUM",
                             side="right"))

            E12 = [[None] * 12 for _ in range(3)]
            pend = {}   # wv tiles awaiting their deferred transpose
            cp_rr = [0]
            att_pool = [None]
            wv_bufs = [1]
            pp_o = None

            def emit_scores(h, kt):
                pk = PT[kt]
                wq = T if h < 2 else 750
                ps = sc_pool[0].tile([128, wq], f32, tag="psc", name="psc")
                sub = FULLSUB if h < 2 else SSUB
                if h < 2:
                    lh = K12[64 * h:64 * h + 64, PT_OFF[kt]:PT_OFF[kt] + pk]
                    qsrc = Q12[64 * h:64 * h + 64, :]
                else:
                    lh = K3[0:64, PT_OFF[kt]:PT_OFF[kt] + pk]
                    qsrc = QS[0:64, :]
                for (o, w) in sub:
                    nc.tensor.matmul(
                        ps[0:pk, o:o + w], lh, qsrc[:, o:o + w],
                        start=True, stop=True,
                    )
                if h < 2:
                    E = epool.tile([128, T], bf16, tag="E", name="E")
                else:
                    E = epool.tile([128, 750], bf16, tag="ES", bufs=12,
                                   name="ES")
                nc.scalar.activation(E[0:pk, 0:wq], ps[0:pk, 0:wq], EXP)
                E12[h][kt] = E

            PRODS = ((XH, 0), (XH, 1), (XL, 0))  # (x level, w level)

            def emit_vproj(tt):
                pk = PT[tt]
                ps = pp_v.tile([128, VW], f32, tag="pv", name="pv")
                n = 0
                for (XS, wl) in PRODS:
                    WT = WVWH if wl == 0 else WVWL
                    for j in range(5):
                        lhsT = ap3(XS[j][:, :], [(XSTR, 2), (1, pk)],
                                   off=PT_OFF[tt])
                        rhs = ap3(WT[:, :], [(VW, 2), (1, VW)],
                                  off=2 * VW * j)
                        nc.tensor.matmul(
                            ps[0:pk, :], lhsT, rhs,
                            start=(n == 0), stop=(n == 14), perf_mode=DR,
                        )
                        n += 1
                nc.vector.scalar_tensor_tensor(
                    V[0:pk, VW * tt:VW * (tt + 1)], ps[0:pk, :], UNSCALE,
                    BV[0:pk, :], AluOpType.mult, AluOpType.add)

            def emit_wv(h, qt):
                """wv for head h, q-tile qt (kt-inner accumulation) plus the
                DVE normalize; the PE transpose is deferred to flush_norm."""
                if h < 2:
                    pkq, qo = PT[qt], PT_OFF[qt]
                else:
                    pkq, qo = SQT, SQT * qt
                ps = att_pool[0].tile([128, VSLOT], f32, tag="wv",
                                      bufs=wv_bufs[0], name="wv")
                for kt in range(12):
                    pkk = PT[kt]
                    if h < 2:
                        vsrc = V[0:pkk, VW * kt + VSLOT * h:
                                 VW * kt + VSLOT * h + VSLOT]
                    else:
                        vsrc = V3[0:pkk, VSLOT * kt:VSLOT * (kt + 1)]
                    nc.tensor.matmul(
                        ps[0:pkq, 0:VSLOT],
                        E12[h][kt][0:pkk, qo:qo + pkq],
                        vsrc,
                        start=(kt == 0), stop=(kt == 11),
                    )
                rz = fin.tile([128, 1], f32, tag="rz", name="rz")
                nc.vector.reciprocal(rz[0:pkq, 0:1], ps[0:pkq, 64:65])
                wsb = fin.tile([128, 64], bf16, tag="wsb", name="wsb")
                nc.vector.tensor_scalar_mul(wsb[0:pkq, 0:64],
                                            ps[0:pkq, 0:64], rz[0:pkq, 0:1])
                pend[(h, qt)] = (wsb, pkq, qo)

            def flush_norm(h, qt):
                """PE transpose + WVT copy for a pending wv tile."""
                wsb, pkq, qo = pend.pop((h, qt))
                pst = att_pool[0].tile([64, 128], bf16, tag="pt", name="pt")
                nc.tensor.transpose(pst[0:64, 0:pkq], wsb[0:pkq, 0:64],
                                    IDN[0:pkq, 0:pkq])
                # h=0/2 run inside exp windows: keep ACT free for exp.
                # h=1 runs in the drain where ACT has slack.
                cp = nc.scalar.copy if h == 1 else nc.vector.tensor_copy
                if h < 2:
                    cp(WVT12[64 * h:64 * h + 64, qo:qo + pkq],
                       pst[0:64, 0:pkq])
                else:
                    cp(WVT3[0:64, qo:qo + pkq], pst[0:64, 0:pkq])

            def emit_outproj(lhs, wo, dst, pk, dsl):
                stage = ost.tile([128, F], bf16, tag="stage", name="stage")
                ps = pp_o.tile([128, 1024], f32, tag="po", name="po")
                pst2 = att_pool[0].tile([128, 256], f32, tag="po256",
                                        name="po256")
                for (o, w) in ((0, 512), (512, 512)):
                    nc.tensor.matmul(
                        ps[0:pk, o:o + w], lhs, wo[:, o:o + w],
                        start=True, stop=True,
                    )
                nc.tensor.matmul(pst2[0:pk, 0:256], lhs, wo[:, 1024:1280],
                                 start=True, stop=True)
                cp_rr[0] += 1
                a, b = ((nc.scalar.copy, nc.vector.tensor_copy)
                        if cp_rr[0] % 2 else
                        (nc.vector.tensor_copy, nc.scalar.copy))
                a(stage[0:pk, 0:1024], ps[0:pk, 0:1024])
                b(stage[0:pk, 1024:1280], pst2[0:pk, 0:256])
                nc.sync.dma_start(dst[dsl, :], stage[0:pk, :])

            def emit_outproj12(tt):
                pk = PT[tt]
                tsl = slice(PT_OFF[tt], PT_OFF[tt] + pk)
                emit_outproj(WVT12[:, tsl], WO12, out_d, pk, tsl)

            def emit_outproj3(qt):
                qsl = slice(SQT * qt, SQT * (qt + 1))
                emit_outproj(WVT3[:, qsl], WO3, out2_d, SQT, qsl)

            # ---- window S: shared-head scores/exp, projection fillers ----
            for kt in range(12):
                emit_scores(2, kt)

            # m0 (q) + m1's FIRST chunk: res-3 DoubleRow in two passes
            # matching the x DMA order (Xh products chase the xh chunks,
            # then the Xl product chases xl).  Riding m1-c0 inside the
            # x-chase means k chunk 0 (and F0's first score tiles) is
            # ready right when the last xl chunk lands instead of two
            # chunk-times later.
            ps0 = [pp_qk.tile([128, 512], f32, tag="pqk", name="pqk",
                              bufs=4) for _ in range(3)]
            ps1c0 = pp_qk.tile([128, 512], f32, tag="pqk", name="pqk",
                               bufs=4)
            n = 0
            for pr in (((XH, 0), (XH, 1)), ((XL, 0),)):
                for j in range(5):
                    for (XS, wl) in pr:
                        WT = WQKH if wl == 0 else WQKL
                        lhsT = ap3(WT[:, :], [(256, 2), (1, 128)],
                                   off=512 * j)
                        for ci, (o, w) in enumerate(FULLSUB):
                            rhs = ap3(XS[j][:, :], [(XSTR, 2), (1, w)],
                                      off=o)
                            nc.tensor.matmul(ps0[ci][0:128, 0:w], lhsT, rhs,
                                             start=(n == 0), stop=(n == 14),
                                             perf_mode=DR)
                        lhsTk = ap3(WT[:, :], [(256, 2), (1, 128)],
                                    off=512 * j + 128)
                        rhsk = ap3(XS[j][:, :], [(XSTR, 2), (1, 512)],
                                   off=0)
                        nc.tensor.matmul(ps1c0[0:128, 0:512], lhsTk, rhsk,
                                         start=(n == 0), stop=(n == 14),
                                         perf_mode=DR)
                        n += 1
            nc.scalar.activation(K12[:, 0:512], ps1c0[0:128, 0:512],
                                 IDF, scale=UNSCALE)
            for ci, (o, w) in enumerate(FULLSUB):
                nc.scalar.activation(Q12[:, o:o + w], ps0[ci][0:128, 0:w],
                                     IDF, bias=BQK[:, 0:1], scale=UNSCALE)

            # m1 (k) remaining chunks: C-MAJOR res-3 with an inline copy
            # per chunk
            for ci, (o, w) in list(enumerate(FULLSUB))[1:]:
                ps1 = pp_qk.tile([128, 512], f32, tag="pqk", name="pqk",
                                 bufs=4)
                n = 0
                for (XS, wl) in PRODS:
                    WT = WQKH if wl == 0 else WQKL
                    for j in range(5):
                        lhsT = ap3(WT[:, :], [(256, 2), (1, 128)],
                                   off=512 * j + 128)
                        rhs = ap3(XS[j][:, :], [(XSTR, 2), (1, w)],
                                  off=o)
                        nc.tensor.matmul(ps1[0:128, 0:w], lhsT, rhs,
                                         start=(n == 0), stop=(n == 14),
                                         perf_mode=DR)
                        n += 1
                # k has no bias; the copy folds the fp8 pre-scale away
                nc.scalar.activation(K12[:, o:o + w], ps1[0:128, 0:w],
                                     IDF, scale=UNSCALE)
            qk_ctx.close()

            # ---- window F0: scores/exp + v-proj fillers ----
            sc_ctx.close()
            sc_ctx = ExitStack()
            sc_pool[0] = sc_ctx.enter_context(
                tc.tile_pool(name="pp_sc", bufs=2, space="PSUM"))
            vp_ctx = ExitStack()
            pp_v = vp_ctx.enter_context(
                tc.tile_pool(name="pp_v", bufs=2, space="PSUM",
                             side="right"))
            for kt in range(12):
                emit_scores(0, kt)
                emit_vproj(kt)
            vp_ctx.close()

            # ---- window F1: scores/exp + F0 wv + shared-head wv ----
            att_pool[0] = att_ctx.enter_context(
                tc.tile_pool(name="pp_att", bufs=1, space="PSUM",
                             side="right"))
            for kt in range(12):
                emit_scores(1, kt)
                emit_wv(0, kt)
                if kt % 2 == 1:
                    emit_wv(2, kt // 2)
                if kt >= 1:
                    flush_norm(0, kt - 1)
                if kt % 2 == 0 and kt >= 2:
                    flush_norm(2, kt // 2 - 1)
            flush_norm(0, 11)
            flush_norm(2, 5)

            # ---- drain: F1 wv + all out-projections ----
            sc_ctx.close()
            att_ctx.close()
            att_ctx = ExitStack()
            att_pool[0] = att_ctx.enter_context(
                tc.tile_pool(name="pp_att2", bufs=1, space="PSUM",
                             side="right"))
            wv_bufs[0] = 2
            o_ctx = ExitStack()
            pp_o = o_ctx.enter_context(
                tc.tile_pool(name="pp_o", bufs=2, space="PSUM"))
            # out2's projections first: WVT3 is complete at F1 end, so
            # their stage copies give ACT/DVE work at drain start
            for qt in range(6):
                emit_outproj3(qt)
            for qt in range(12):
                emit_wv(1, qt)
                if qt >= 1:
                    flush_norm(1, qt - 1)
            flush_norm(1, 11)
            for tt in range(12):
                emit_outproj12(tt)
            o_ctx.close()
            att_ctx.close()

    nc.compile()
    return nc


def _get_nc(_unused=None):
    if "nc" not in _CACHE:
        _CACHE["nc"] = _build()
    return _CACHE["nc"]


def _split8(a):
    import ml_dtypes
    a = np.asarray(a, dtype=np.float32)
    hi = a.astype(ml_dtypes.float8_e4m3)
    lo = (a - hi.astype(np.float32)).astype(ml_dtypes.float8_e4m3)
    return hi, lo


def _to_bf(a):
    import ml_dtypes
    return np.ascontiguousarray(np.asarray(a, dtype=np.float32)).astype(
        ml_dtypes.bfloat16)


def _prep_all(x, Wq, bq, Wk, Wv, bv, Wo):
    x = np.asarray(x, dtype=np.float32).reshape(T, F)
    xT = np.ascontiguousarray(x.T)
    xh, xl = _split8(xT * np.float32(8.0))
    WqT = np.ascontiguousarray(np.asarray(Wq, dtype=np.float32).T)
    WkTs = (np.ascontiguousarray(np.asarray(Wk, dtype=np.float32).T)
            * np.float32(QK_SCALE))
    WvT = np.ascontiguousarray(np.asarray(Wv, dtype=np.float32).T)
    WoT = np.ascontiguousarray(np.asarray(Wo, dtype=np.float32).T)
    bq = np.asarray(bq, dtype=np.float32)
    bvv = np.asarray(bv, dtype=np.float32)

    def hsl(h):
        return slice(D * h, D * (h + 1))

    in_maps = []
    for c in range(NCORES):
        F0, F1 = 2 * c, 2 * c + 1
        S = 16 + c // 2

        wqk = np.zeros((F, 256), dtype=np.float32)
        wqk[:, 0:64] = WqT[:, hsl(F0)]
        wqk[:, 64:128] = WqT[:, hsl(F1)]
        wqk[:, 128:192] = WkTs[:, hsl(F0)]
        wqk[:, 192:256] = WkTs[:, hsl(F1)]
        wqkh, wqkl = _split8(wqk * np.float32(128.0))

        bqk = np.zeros((128, 1), dtype=np.float32)
        bqk[0:64, 0] = bq[hsl(F0)]
        bqk[64:128, 0] = bq[hsl(F1)]

        roff = 750 * (c % 2)
        qs = (x[roff:roff + 750] @ WqT[:, hsl(S)] + bq[hsl(S)]).T
        k3 = (x @ WkTs[:, hsl(S)]).T
        v3 = np.zeros((1536, VSLOT), dtype=np.float32)
        v3[0:T, 0:64] = x @ WvT[:, hsl(S)] + bvv[hsl(S)]
        v3[0:T, 64] = 1.0

        wvw = np.zeros((F, VW), dtype=np.float32)
        bvr = np.zeros((1, VW), dtype=np.float32)
        for s, h in enumerate((F0, F1)):
            wvw[:, VSLOT * s:VSLOT * s + 64] = WvT[:, hsl(h)]
            bvr[0, VSLOT * s:VSLOT * s + 64] = bvv[hsl(h)]
            bvr[0, VSLOT * s + 64] = 1.0
        wvwh, wvwl = _split8(wvw * np.float32(128.0))

        wo = np.zeros((192, F), dtype=np.float32)
        wo[0:64] = WoT[hsl(F0), :]
        wo[64:128] = WoT[hsl(F1), :]
        wo[128:192] = WoT[hsl(S), :]

        idn = np.eye(128, dtype=np.float32)

        in_maps.append({
            "xh": xh, "xl": xl,
            "wqkh": wqkh, "wqkl": wqkl,
            "bqk": bqk,
            "qs": _to_bf(qs),
            "k3": _to_bf(k3),
            "v3": _to_bf(v3),
            "wvwh": wvwh, "wvwl": wvwl,
            "bv": bvr,
            "wo": _to_bf(wo),
            "idn": _to_bf(idn),
        })
    return in_maps


def _make_runner(nc):
    """Axon-path runner (built once, reused)."""
    import jax
    import jax.numpy as jnp
    import concourse.mybir as mybir
    from concourse import bass2jax
    from jax.experimental.shard_map import shard_map
    from jax.sharding import Mesh, PartitionSpec

    bass2jax.install_neuronx_cc_hook()

    partition_name = (nc.partition_id_tensor.name
                      if nc.partition_id_tensor else None)

    REPLICATED = {"xh", "xl", "idn"}
    in_names, out_names, out_avals, zero_templates = [], [], [], []
    for alloc in nc.m.functions[0].allocations:
        if not isinstance(alloc, mybir.MemoryLocationSet):
            continue
        name = alloc.memorylocations[0].name
        if alloc.kind == "ExternalInput":
            if name != partition_name:
                in_names.append(name)
        elif alloc.kind == "ExternalOutput":
            out_names.append(name)
            shape = tuple(alloc.tensor_shape)
            dtype = mybir.dt.np(alloc.dtype)
            out_avals.append(jax.core.ShapedArray(shape, dtype))
            zero_templates.append((shape, dtype))
    n_params = len(in_names)
    n_outs = len(out_avals)
    all_names = in_names + out_names
    if partition_name is not None:
        all_names = all_names + [partition_name]
    donate = tuple(range(n_params, n_params + n_outs))
    i_out = out_names.index("out")
    i_out2 = out_names.index("out2")

    devices = jax.devices()[:NCORES]
    mesh = Mesh(np.asarray(devices), ("core",))

    def _body(*args):
        operands = list(args)
        if partition_name is not None:
            operands.append(bass2jax.partition_id_tensor())
        outs = bass2jax._bass_exec_p.bind(
            *operands,
            out_avals=tuple(out_avals),
            in_names=tuple(all_names),
            out_names=tuple(out_names),
            lowering_input_output_aliases=(),
            sim_require_finite=True,
            sim_require_nnan=True,
            nc=nc,
        )
        return tuple(outs)

    in_specs = tuple(
        PartitionSpec() if n in REPLICATED else PartitionSpec("core")
        for n in in_names
    ) + (PartitionSpec("core"),) * n_outs
    bass_fn = jax.jit(
        shard_map(_body, mesh=mesh, in_specs=in_specs,
                  out_specs=(PartitionSpec("core"),) * n_outs,
                  check_rep=False),
        donate_argnums=donate, keep_unused=True,
    )

    def _zeros():
        return tuple(jnp.zeros(s, d) for (s, d) in zero_templates)

    zeros_fn = jax.jit(
        shard_map(_zeros, mesh=mesh, in_specs=(),
                  out_specs=(PartitionSpec("core"),) * n_outs,
                  check_rep=False))

    def _combine(o, o2):
        idx = jax.lax.axis_index("core")
        off = 750 * (idx % 2)
        o = o.astype(jnp.float32)
        z = jnp.zeros((T, F), jnp.float32)
        z = jax.lax.dynamic_update_slice(
            z, o2[0:750].astype(jnp.float32), (off, 0))
        return jax.lax.psum(o + z, "core")

    reduce_fn = jax.jit(
        shard_map(_combine, mesh=mesh,
                  in_specs=(PartitionSpec("core"), PartitionSpec("core")),
                  out_specs=PartitionSpec(), check_rep=False))

    dev_cache = {}

    def run(in_maps):
        args = []
        for n in in_names:
            if n in REPLICATED:
                arr = np.asarray(in_maps[0][n])
            else:
                arr = np.concatenate(
                    [np.asarray(in_maps[c][n]) for c in range(NCORES)],
                    axis=0)
            fp = (arr.shape, hash(arr.tobytes()))
            cached = dev_cache.get(n)
            if cached is not None and cached[0] == fp:
                args.append(cached[1])
            else:
                dev_arr = jax.device_put(
                    arr, jax.sharding.NamedSharding(
                        mesh,
                        PartitionSpec() if n in REPLICATED
                        else PartitionSpec("core")))
                dev_cache[n] = (fp, dev_arr)
                args.append(dev_arr)
        zeros = zeros_fn()
        outs = bass_fn(*args, *zeros)
        total = reduce_fn(outs[i_out], outs[i_out2])
        return np.asarray(total)

    return run


def kernel(x, Wq, bq, Wk, Wv, bv, Wo, bo):
    global LAST_RESULTS

    nc = _get_nc()
    in_maps = _prep_all(x, Wq, bq, Wk, Wv, bv, Wo)

    from concourse._compat import axon_active

    if axon_active():
        key = "runner"
        if key not in _CACHE:
            _CACHE[key] = _make_runner(nc)
        out = np.array(_CACHE[key](in_maps), dtype=np.float32)
    else:
        from concourse.bass_utils import run_bass_kernel_spmd
        trace = os.environ.get("KERNEL_TRACE", "0") == "1"
        res = run_bass_kernel_spmd(nc, in_maps, core_ids=list(range(NCORES)),
                                   trace=trace)
        LAST_RESULTS = res
        out = np.zeros((T, F), dtype=np.float32)
        for c in range(NCORES):
            out += np.asarray(res.results[c]["out"], dtype=np.float32)
            roff = 750 * (c % 2)
            out[roff:roff + 750] += np.asarray(res.results[c]["out2"],
                                               dtype=np.float32)
    out += np.asarray(bo, dtype=np.float32)
    return out.reshape(1, T, F)
